# revision 12
# baseline (speedup 1.0000x reference)
"""Trainium2 Bass kernel for nn_MaxMarginLoss (segment_reduce).

Reference computation (B=16, T=2048, D=1024, S=32, ALPHA=1):
    x = |inputs|                                   [B, T, D]
    H[b, s] = mean_{t: step_ids[b,t]==s+1} x[b,t]  [B, S, D]
    E[b, s] = sum_d relu(H[b,s,d] - H[b,s+1,d])^2  [B, S-1]
    loss_pos = mean_s E ; loss_neg = mean_s relu(1 - E)
    per_sample = label ? loss_pos : loss_neg ; out = sum_b / 16

Sharding: data-parallel over B — 2 samples per core on 8 cores.
Device does: abs + segment-mean (as onehot matmul) + energy tail;
host does: onehot construction from step_ids (tiny int work) and the
final 8-way partial-sum gather (8 floats).
"""

import sys

if "/opt/trn_rl_repo" not in sys.path:
    sys.path.insert(0, "/opt/trn_rl_repo")

import numpy as np

import bass_rust as _bass_rust
import concourse.bass as bass
import concourse.tile as tile
from concourse import mybir
from concourse.bass_utils import run_bass_kernel_spmd

B, T, D, S = 16, 2048, 1024, 32
ALPHA = 1.0
NCORES = 8
NSAMP = B // NCORES          # samples per core
P = 128                      # partitions
KT = T // P                  # 16 k-tiles per sample
LOAD_KT = 2                  # k-tiles per DMA load (1 MiB loads)
F32 = mybir.dt.float32
Alu = mybir.AluOpType


def _build_bass():
    nc = bass.Bass()
    x_in = nc.dram_tensor("x", [NSAMP, T, D], F32, kind="ExternalInput")
    w_in = nc.dram_tensor("w", [NSAMP, P, KT * S], F32, kind="ExternalInput")
    wrow_in = nc.dram_tensor("wrow", [1, 2 * NSAMP], F32, kind="ExternalInput")
    out_d = nc.dram_tensor("out", [1, 1], F32, kind="ExternalOutput")

    with tile.TileContext(nc) as tc:
        with (
            # one buffer per load tile (16 x 1MiB = 128KB/partition):
            # zero slot reuse means zero WAR waits, so every DMA carries
            # only its queue wait and every compute op a single RAW wait
            tc.tile_pool(name="xload", bufs=(NSAMP * KT) // LOAD_KT) as xload,
            tc.tile_pool(name="wpool", bufs=2) as wpool,
            tc.tile_pool(name="small", bufs=1) as small,
            tc.tile_pool(name="tailp", bufs=2) as tailp,
            tc.tile_pool(name="hpsum", bufs=2, space="PSUM") as hpsum,
            tc.tile_pool(name="rpsum", bufs=1, space="PSUM") as rpsum,
        ):
            # Wait-count discipline (TRN2 walrus limits: DMA allows 1 dep
            # wait + queue wait; fp32 self-loading LDWEIGHTS allows 1):
            # every DMA'd tile has exactly ONE consumer engine, and DMA'd
            # operands of multi-input compute ops are pre-copied through
            # the engine that consumes them.
            ones31 = small.tile([S - 1, 1], F32)
            nc.vector.memset(ones31, 1.0)
            wrow_raw = small.tile([1, 2 * NSAMP], F32, tag="wrow_raw")
            nc.sync.dma_start(out=wrow_raw, in_=wrow_in[:, :])
            wrow_sb = small.tile([1, 2 * NSAMP], F32, tag="wrow")
            nc.vector.tensor_copy(wrow_sb, wrow_raw)
            # per-sample [sum(E), sum(min(E-1,0))] columns
            ef = small.tile([S - 1, 2 * NSAMP], F32)

            for i in range(NSAMP):
                w_raw = wpool.tile([P, KT * S], F32, tag="wraw")
                nc.sync.dma_start(out=w_raw, in_=w_in[i])
                w_sb = wpool.tile([P, KT * S], F32, tag="w")
                nc.vector.tensor_copy(w_sb, w_raw)
                # H accumulator [S, 2, 512]: two PSUM banks, one per D half
                hp = hpsum.tile([S, 2, 512], F32, tag="hp")

                x_i = x_in[i].rearrange("(kk j p) d -> kk p j d", j=LOAD_KT, p=P)
                for kk in range(KT // LOAD_KT):
                    xt = xload.tile([P, LOAD_KT, D], F32, tag="xt")
                    nc.sync.dma_start(out=xt, in_=x_i[kk])
                    # |x| in place on ScalarE (ACT table Abs)
                    nc.scalar.activation(
                        xt[:, :, :], xt[:, :, :],
                        mybir.ActivationFunctionType.Abs,
                    )
                    for j in range(LOAD_KT):
                        k = kk * LOAD_KT + j
                        for ds in range(2):
                            nc.tensor.matmul(
                                out=hp[:, ds, :],
                                lhsT=w_sb[:, k * S:(k + 1) * S],
                                rhs=xt[:, j, ds * 512:(ds + 1) * 512],
                                start=(k == 0),
                                stop=(k == KT - 1),
                            )

                # tail: E[s] = sum_d relu(H[s]-H[s+1])^2, all on DVE so
                # cross-engine fan-in stays at one wait per instruction
                h_sb = tailp.tile([S, D], F32, tag="h")
                nc.vector.tensor_copy(h_sb, hp[:, :, :])
                hs_sb = tailp.tile([S - 1, D], F32, tag="hs")
                # partition shift via SBUF->SBUF DMA: hs[p] = H[p+1]
                nc.sync.dma_start(out=hs_sb, in_=h_sb[1:S, :])
                d_t = tailp.tile([S - 1, D], F32, tag="d")
                nc.vector.tensor_sub(d_t, h_sb[0:S - 1, :], hs_sb[:, :])
                nc.vector.tensor_scalar_max(d_t, d_t, 0.0)
                sq_t = tailp.tile([S - 1, D], F32, tag="sq")
                nc.vector.tensor_mul(sq_t, d_t, d_t)
                nc.vector.tensor_reduce(
                    out=ef[:, 2 * i:2 * i + 1],
                    in_=sq_t,
                    axis=mybir.AxisListType.X,
                    op=Alu.add,
                )
                # F = min(E-1, 0) = -relu(1 - E)
                nc.vector.tensor_scalar(
                    ef[:, 2 * i + 1:2 * i + 2],
                    ef[:, 2 * i:2 * i + 1],
                    1.0,
                    0.0,
                    Alu.subtract,
                    Alu.min,
                )

            # cross-partition sum of ef columns, then dot with wrow
            red = rpsum.tile([1, 2 * NSAMP], F32)
            nc.tensor.matmul(out=red, lhsT=ones31, rhs=ef, start=True, stop=True)
            sc = small.tile([1, 2 * NSAMP], F32, tag="sc")
            acc = small.tile([1, 1], F32, tag="acc")
            nc.vector.tensor_mul(sc, red[:, :], wrow_sb[:, :])
            nc.vector.tensor_reduce(
                out=acc, in_=sc, axis=mybir.AxisListType.X, op=Alu.add
            )
            nc.sync.dma_start(out=out_d[:, :], in_=acc)

    # TRN2 walrus enforces per-instruction sync-wait limits (1 for
    # LDWEIGHTS/DMA-2D). These bacc passes split excess waits into
    # event-semaphore chains; plain Bass.finalize doesn't run them.
    _bass_rust.move_matmul_waits_to_ldweights(nc.m)
    _bass_rust.generate_event_semaphores(nc)
    return nc


_NC = None


def _get_nc():
    global _NC
    if _NC is None:
        _NC = _build_bass()
    return _NC


def _make_in_maps(inputs, step_ids, binary_labels):
    x = np.ascontiguousarray(np.asarray(inputs, dtype=np.float32))
    sid = np.asarray(step_ids)
    lab = np.asarray(binary_labels)

    onehot = (sid[..., None] == np.arange(1, S + 1)).astype(np.float32)  # [B,T,S]
    counts = onehot.sum(axis=1)  # [B,S]
    w = onehot / counts[:, None, :]
    # SBUF layout: w_pre[b, p, k*S + c] = w[b, k*128 + p, c]
    w_pre = np.ascontiguousarray(
        w.reshape(B, KT, P, S).transpose(0, 2, 1, 3).reshape(B, P, KT * S)
    )
    wpos = (lab == 1).astype(np.float32)

    in_maps = []
    for c in range(NCORES):
        b0 = c * NSAMP
        wrow = np.zeros((1, 2 * NSAMP), np.float32)
        for i in range(NSAMP):
            wrow[0, 2 * i] = wpos[b0 + i] / np.float32(S - 1)
            wrow[0, 2 * i + 1] = -(1.0 - wpos[b0 + i]) / np.float32(S - 1)
        in_maps.append(
            {
                "x": x[b0:b0 + NSAMP],
                "w": w_pre[b0:b0 + NSAMP],
                "wrow": wrow,
            }
        )
    return in_maps


def _combine(results):
    total = np.float32(0.0)
    for r in results:
        total = np.float32(total + np.float32(r["out"][0, 0]))
    return np.asarray(total / np.float32(16.0), dtype=np.float32)


def kernel(inputs, step_ids, binary_labels):
    in_maps = _make_in_maps(inputs, step_ids, binary_labels)
    nc = _get_nc()
    res = run_bass_kernel_spmd(nc, in_maps, list(range(NCORES)))
    return _combine(res.results)


def kernel_profiled(inputs, step_ids, binary_labels, **kwargs):
    """Like kernel() but returns (out, BassKernelResults) with trace."""
    in_maps = _make_in_maps(inputs, step_ids, binary_labels)
    nc = _get_nc()
    res = run_bass_kernel_spmd(
        nc, in_maps, list(range(NCORES)), trace=True, **kwargs
    )
    return _combine(res.results), res
